# revision 24
# baseline (speedup 1.0000x reference)
"""Trainium2 Bass kernel for nn_AttentionModule (B=8, C=256, HID=32, H=W=64).

Data-parallel over batch: each of the 8 NeuronCores computes one batch
element's full attention:
    q = wq @ xf + bq            [32, 4096]
    k = wk @ xf                 [32, 4096]   (bk dropped: it shifts every
                                 score in a softmax row by the same constant,
                                 so softmax is invariant to it — exact)
    scores^T[j, i] = sum_d k[d, j] q[d, i]
    attn = softmax over j (no max subtraction: |scores| <= ~45, exp fits bf16)
    out[i, c] = (sum_j exp(scoresT[j, i]) vT[j, c]) / (sum_j exp(scoresT[j, i]))

Layout choices:
 - scores^T is computed in [j(partition), i(free)] orientation so that the
   A@V matmul needs no transposes (contraction j on partitions for both
   operands).
 - The softmax denominator comes free as an all-ones 257th column of vT.
 - Output is produced in [i(partition), c(free)] orientation and DMA'd to a
   [N, C] dram tensor; the host transposes back to [C, N] (free on CPU).
 - q/k are replicated 4x along partitions (qstack/kstack [128, 4096]) so the
   K=32 score matmuls can be issued to 4 distinct PE row-groups
   (tile_position) and run concurrently.
 - Exp groups are TGRP=8 j-chunks with a double-buffered PSUM score pool, so
   the PE never waits for the ACT exp of the previous group.
"""

import os
import sys
import math
import functools
from contextlib import ExitStack

import numpy as np

for _p in ("/opt/trn_rl_repo", os.path.expanduser("~/.axon_site/_ro/trn_rl_repo")):
    if os.path.isdir(_p) and _p not in sys.path:
        sys.path.insert(0, _p)

import ml_dtypes  # noqa: E402

import concourse.bass as bass  # noqa: E402
import concourse.tile as tile  # noqa: E402
from concourse import bacc  # noqa: E402
from concourse import mybir  # noqa: E402

B, C, HID, H, W = 8, 256, 32, 64, 64
N = H * W  # 4096
N_CORES = 8

F32 = mybir.dt.float32
F32R = mybir.dt.float32r
BF16 = mybir.dt.bfloat16
F16 = mybir.dt.float16
F8E4 = mybir.dt.float8e4
BF = ml_dtypes.bfloat16

JC = 128                      # j-chunk height (partition dim of scores^T)
N_JCHUNK = N // JC            # 32
TIT = 128                     # i-tile width
N_IT = N // TIT               # 32
TGRP = int(os.environ.get("BASS_TGRP", "8"))   # j-chunks per exp group
TNG = N_JCHUNK // TGRP        # groups per i-tile
PSC_BUFS = int(os.environ.get("BASS_PSC_BUFS", "2"))
POT_BUFS = int(os.environ.get("BASS_POT_BUFS", "2"))
EXPP_BUFS = int(os.environ.get("BASS_EXPP_BUFS", "4"))
AV_LAG = int(os.environ.get("BASS_AV_LAG", "2"))
# score-quad packing: concurrent tile_position matmuls must hit DISTINCT PSUM
# banks — one scp tile [128, 8, 128] spans 2 banks, capping in-tile packing at
# pairs (PACK=2). PACK=4 interleaves TWO groups' score matmuls (their psc
# tiles are different pool buffers = 4 distinct banks total): every adjacent
# window of 4 matmuls covers 4 distinct PE row-quadrants AND 4 distinct PSUM
# banks, without changing the PSUM footprint.
PACK = int(os.environ.get("BASS_PACK", "4"))
# timing probes (break correctness, keep the instruction mix realistic):
#   "scale0"   — exp(0*s)=1: sanity, should time like the real kernel
#   "f8dr"     — scale-0 exp to fp8e4 + DoubleRow AV with main+residual fp8 vT
#   "skipexp"  — exp only on even groups; odd groups reuse the previous eg.
#                PE stream identical, ACT halved: isolates the PE HW factor.
#   "avnarrow" — AV rhs only 128 wide (PE av halved), ACT unchanged:
#                isolates the ACT HW factor.
TPROBE = os.environ.get("BASS_TPROBE", "")
# feed QKV matmuls x directly as bitcast float32r (no f16 conversion pass)
XF32R = os.environ.get("BASS_XF32R", "0") == "1"
# dummy matmuls at t=0 to ramp the PE p-state while waiting for the x DMA
WARMMM = int(os.environ.get("BASS_WARMMM", "0"))


def build_nc(repeat=None):
    nc = _build_inner(repeat)
    nc.compile()
    return nc


def _build_inner(repeat=None):
    """Build the single-core Bass program (run SPMD on 8 cores)."""
    if repeat is None:
        repeat = 1
    nc = bacc.Bacc("TRN2", target_bir_lowering=False, debug=False)

    x_d = nc.dram_tensor("x", [C, N], F32, kind="ExternalInput").ap()
    wq4_d = nc.dram_tensor("wq4", [128, 2, 128], F16,
                           kind="ExternalInput").ap()
    wk4_d = nc.dram_tensor("wk4", [128, 2, 128], F16,
                           kind="ExternalInput").ap()
    bq_d = nc.dram_tensor("bq", [128, 1], F32, kind="ExternalInput").ap()
    wvT_d = nc.dram_tensor("wvT", [128, 2, C], F16, kind="ExternalInput").ap()
    out_d = nc.dram_tensor("out", [N, C], F32, kind="ExternalOutput").ap()

    with tile.TileContext(nc) as tc, ExitStack() as ctx:
        const = ctx.enter_context(tc.tile_pool(name="const", bufs=1))
        stage = ctx.enter_context(tc.tile_pool(name="stage", bufs=1))
        big = ctx.enter_context(tc.tile_pool(name="big", bufs=1))
        expp = ctx.enter_context(tc.tile_pool(name="expp", bufs=EXPP_BUFS))
        outp = ctx.enter_context(tc.tile_pool(name="outp", bufs=3))

        # ---- weights first on sync (small), then x sliced on both queues -----
        # x chunks [128, N] f32, each split into 4 column slices so the first
        # QKV matmul starts after ~1/4 of a chunk lands; f32->f16 conversion
        # per slice alternating DVE/ACT.
        # preload the ACT exp table while the x DMA is in flight (the
        # first real exp would otherwise eat the ~1.3us table load mid-ramp)
        warm = const.tile([1, 1], F32)
        nc.vector.memset(warm, 0.0)
        warme = const.tile([1, 1], BF16)
        nc.scalar.activation(out=warme, in_=warm,
                             func=mybir.ActivationFunctionType.Exp)

        wq4 = const.tile([128, 2, 128], F16)
        nc.sync.dma_start(out=wq4, in_=wq4_d)
        wk4 = const.tile([128, 2, 128], F16)
        nc.sync.dma_start(out=wk4, in_=wk4_d)
        bq = const.tile([128, 1], F32)
        nc.sync.dma_start(out=bq, in_=bq_d)
        wvT = const.tile([128, 2, C], F16)
        nc.gpsimd.dma_start(out=wvT, in_=wvT_d)

        # x [128, N] f32 in column slices over 3 DMA queues, balanced to
        # ~11KB/partition each and ordered by first-use: short head slices
        # so the first QKV matmuls start early, tails cross-assigned so no
        # queue carries more than ~1/3 of the 32KB/partition total.
        SLICES = [(0, 512), (512, 512), (1024, 1024), (2048, 2048)]
        # (ch, si) -> queue: own-chunk queue, slice-1s ride the ACT queue
        QMAP = {(0, 0): "s", (0, 1): "a", (0, 2): "s", (0, 3): "s",
                (1, 0): "g", (1, 1): "a", (1, 2): "g", (1, 3): "g"}
        xqk = None if XF32R else big.tile([128, 2, N], F16, tag="xqk")
        xs_tiles = []
        order = sorted(QMAP, key=lambda t: (t[1], t[0]))
        for ch, si in order:
            off, w = SLICES[si]
            xs = stage.tile([128, w], F32, name=f"xs{ch}_{si}")
            eng = {"s": nc.sync, "g": nc.gpsimd, "a": nc.scalar}[QMAP[(ch, si)]]
            eng.dma_start(
                out=xs,
                in_=x_d[128 * ch:128 * (ch + 1), off:off + w])
            xs_tiles.append((ch, si, off, w, xs))

        def x_ap(ch, col0, width):
            """AP for x[ch-chunk, col0:col0+width] as a matmul operand."""
            if not XF32R:
                return xqk[:, ch, col0:col0 + width]
            for xch, si, off, w, xs in xs_tiles:
                if xch == ch and off <= col0 and col0 + width <= off + w:
                    return xs[:, col0 - off:col0 - off + width].bitcast(F32R)
            raise AssertionError((ch, col0, width))

        if not XF32R:
            # convert slice-major so both chunks' early columns land first;
            # on the otherwise-idle Pool engine (SBUF->SBUF is walrus-safe,
            # unlike PSUM-source gpsimd copies) so ACT/DVE stay free for
            # the qkv copies and early exps during the ramp
            for i, (ch, si, off, w, xs) in enumerate(
                    sorted(xs_tiles, key=lambda t: (t[1], t[0]))):
                sl = slice(off, off + w)
                if i % 2 == 0:
                    nc.vector.tensor_copy(xqk[:, ch, sl], xs)
                else:
                    nc.gpsimd.tensor_copy(xqk[:, ch, sl], xs)

        # ---- qstack/kstack [128, N] (q/k replicated 4x on partitions) --------
        # Only k chunks 0-1 + q chunk 0 are emitted before the main loop; the
        # rest stream in as fillers just ahead of the score/AV matmuls that
        # consume them, so the PE never sits through a serial setup phase.
        qstack = big.tile([128, N], F16, tag="qstack")
        kstack = big.tile([128, N], F16, tag="kstack")
        VW = C + 1
        vT = big.tile([128, N_JCHUNK, VW], BF16, tag="vT")
        nc.vector.memset(vT[:, :, C:C + 1], 1.0)
        pqkv = ctx.enter_context(tc.tile_pool(name="pqkv", bufs=2,
                                              space="PSUM"))

        def emit_k(nch):
            ns = bass.ts(nch, 512)
            pk = pqkv.tile([128, 512], F32, tag="pqkv", name=f"pk{nch}")
            nc.tensor.matmul(pk, lhsT=wk4[:, 0, :],
                             rhs=x_ap(0, 512 * nch, 512),
                             start=True, stop=False)
            nc.tensor.matmul(pk, lhsT=wk4[:, 1, :],
                             rhs=x_ap(1, 512 * nch, 512),
                             start=False, stop=True)
            nc.scalar.activation(out=kstack[:, ns], in_=pk,
                                 func=mybir.ActivationFunctionType.Copy)

        def emit_q(nch):
            ns = bass.ts(nch, 512)
            pq = pqkv.tile([128, 512], F32, tag="pqkv", name=f"pq{nch}")
            nc.tensor.matmul(pq, lhsT=wq4[:, 0, :],
                             rhs=x_ap(0, 512 * nch, 512),
                             start=True, stop=False)
            nc.tensor.matmul(pq, lhsT=wq4[:, 1, :],
                             rhs=x_ap(1, 512 * nch, 512),
                             start=False, stop=True)
            nc.vector.tensor_scalar(out=qstack[:, ns], in0=pq,
                                    scalar1=bq, scalar2=None,
                                    op0=mybir.AluOpType.add)

        def emit_v(jc):
            js = bass.ts(jc, 128)
            pv = pqkv.tile([128, C], F32, tag="pqkv", name=f"pv{jc}")
            nc.tensor.matmul(pv, lhsT=x_ap(0, 128 * jc, 128),
                             rhs=wvT[:, 0, :],
                             start=True, stop=False)
            nc.tensor.matmul(pv, lhsT=x_ap(1, 128 * jc, 128),
                             rhs=wvT[:, 1, :],
                             start=False, stop=True)
            nc.vector.tensor_copy(vT[:, jc, 0:C], pv)

        # PE p-state warm-up: the PE would idle ~3.5us here waiting for x,
        # then run its first ~3us of real matmuls at 0.65/1.2GHz (full clock
        # needs 3us of continuous busy). Dummy matmuls on memset data keep it
        # busy through the DMA window at zero cost.
        if WARMMM:
            warmpe = const.tile([128, 512], F16)
            nc.vector.memset(warmpe, 0.0)
            for wi in range(WARMMM):
                pwarm = pqkv.tile([128, 512], F32, tag="pqkv",
                                  name=f"pwarm{wi}")
                nc.tensor.matmul(pwarm, lhsT=warmpe[:, 0:128], rhs=warmpe,
                                 start=True, stop=True)

        emit_k(0)
        emit_k(1)
        emit_q(0)

        SCHED = [(it, g) for it in range(N_IT) for g in range(TNG)]

        # fillers keyed by (rep, it, g): "pre" run before that group's score
        # matmuls (they feed them), "post" after (they feed the NEXT av)
        fill_pre = {}
        fill_post = {}
        fill_pre[(0, 0, 0)] = [lambda: emit_k(2), lambda: emit_k(3)]
        fill_post[(0, 0, 1)] = [lambda jc=jc: emit_v(jc) for jc in range(0, 8)]
        fill_pre[(0, 0, 2)] = [lambda: emit_k(4), lambda: emit_k(5)]
        fill_post[(0, 0, 2)] = [lambda jc=jc: emit_v(jc) for jc in range(8, 16)]
        fill_pre[(0, 0, 3)] = [lambda: emit_k(6), lambda: emit_k(7)]
        fill_post[(0, 0, 3)] = [lambda jc=jc: emit_v(jc) for jc in range(16, 24)]
        fill_post[(0, 1, 0)] = [lambda jc=jc: emit_v(jc) for jc in range(24, 32)]
        for qc in range(1, 8):
            fill_pre[(0, 4 * qc - 1, 0)] = [lambda qc=qc: emit_q(qc)]

        # ---- fp8 probe tiles --------------------------------------------------
        if TPROBE == "f8dr":
            vT8 = big.tile([128, N_JCHUNK, VW], F8E4, tag="vT8")
            vT8r = big.tile([128, N_JCHUNK, VW], F8E4, tag="vT8r")
            for jc in range(N_JCHUNK):
                nc.vector.tensor_copy(vT8[:, jc, :], vT[:, jc, :])
                nc.vector.tensor_copy(vT8r[:, jc, :], vT[:, jc, :])

        # ---- attention main loop (fused denominator, out in [i, c]) ----------
        psc = ctx.enter_context(tc.tile_pool(name="psc", bufs=PSC_BUFS,
                                             space="PSUM"))
        pot = ctx.enter_context(tc.tile_pool(name="pot", bufs=POT_BUFS,
                                             space="PSUM"))
        pots = {}
        pending = []

        def issue_av(p_rep, p_it, p_g, p_eg):
            p_pot = pots[(p_rep, p_it)]
            if TPROBE == "avnarrow":
                for jcl in range(TGRP):
                    jc = p_g * TGRP + jcl
                    nc.tensor.matmul(p_pot[:, 0:128], lhsT=p_eg[:, jcl, :],
                                     rhs=vT[:, jc, 0:128],
                                     start=jc == 0, stop=jc == N_JCHUNK - 1)
            elif TPROBE == "f8dr":
                for half, vt8x in ((0, vT8), (1, vT8r)):
                    for jp in range(TGRP // 2):
                        jc = p_g * TGRP + 2 * jp
                        nc.tensor.matmul(
                            p_pot,
                            lhsT=p_eg[:, 2 * jp:2 * jp + 2, :],
                            rhs=vt8x[:, jc:jc + 2, :],
                            start=jc == 0 and half == 0,
                            stop=jc == N_JCHUNK - 2 and half == 1,
                            perf_mode=mybir.MatmulPerfMode.DoubleRow)
            else:
                for jcl in range(TGRP):
                    jc = p_g * TGRP + jcl
                    nc.tensor.matmul(p_pot, lhsT=p_eg[:, jcl, :],
                                     rhs=vT[:, jc, :],
                                     start=jc == 0, stop=jc == N_JCHUNK - 1)
            if p_g == TNG - 1:
                isl = bass.ts(p_it, TIT)
                rcp = outp.tile([128, 1], F32, tag="rcp",
                                name=f"rcp{p_rep}_{p_it}")
                nc.vector.reciprocal(rcp, p_pot[:, C:C + 1])
                ots = outp.tile([128, C], F32, tag="ots",
                                name=f"ots{p_rep}_{p_it}")
                nc.vector.tensor_scalar(out=ots, in0=p_pot[:, 0:C],
                                        scalar1=rcp, scalar2=None,
                                        op0=mybir.AluOpType.mult)
                nc.sync.dma_start(out=out_d[isl, :], in_=ots)
                del pots[(p_rep, p_it)]

        def emit_scores(pairs):
            """Score matmuls for one or two groups, interleaved so every
            adjacent window of PACK matmuls hits distinct PE row-quadrants
            and distinct PSUM banks. pairs = [(g, isl, scp), ...]."""
            npair = len(pairs)
            for base in range(TGRP // 2):
                for gi in range(npair):
                    g, isl, scp = pairs[gi]
                    for half in range(2):
                        jcl = base + half * (TGRP // 2)
                        jc = g * TGRP + jcl
                        rg = 32 * ((2 * gi + half) % 4)
                        nc.tensor.matmul(
                            scp[:, jcl, :],
                            lhsT=kstack[rg:rg + 32, bass.ts(jc, 128)],
                            rhs=qstack[rg:rg + 32, isl],
                            start=True, stop=True,
                            tile_position=(rg, 0))

        def emit_exp(gg, scp):
            if TPROBE == "skipexp" and (gg % 2 == 1) and pending:
                return pending[-1][3]
            eg = expp.tile([128, TGRP, TIT],
                           F8E4 if TPROBE == "f8dr" else BF16, tag="eg")
            nc.scalar.activation(out=eg, in_=scp,
                                 func=mybir.ActivationFunctionType.Exp,
                                 scale=0.0 if TPROBE else 1.0)
            return eg

        NGG = N_IT * TNG
        STEP = 2 if PACK == 4 else 1
        for gg0 in range(0, NGG * repeat + AV_LAG, STEP):
            batch = []
            for gg in range(gg0, min(gg0 + STEP, NGG * repeat)):
                rep, gg_r = divmod(gg, NGG)
                it, g = SCHED[gg_r]
                for f in fill_pre.pop((rep, it, g), ()):
                    f()
                if g == 0:
                    pots[(rep, it)] = pot.tile([128, C + 1], F32, tag="pot",
                                               name=f"pot{rep}_{it}")
                scp = psc.tile([128, TGRP, TIT], F32, tag="scp")
                batch.append((gg, rep, it, g, scp))
            if batch:
                emit_scores([(g, bass.ts(it, TIT), scp)
                             for (gg, rep, it, g, scp) in batch])
            for gg, rep, it, g, scp in batch:
                eg = emit_exp(gg, scp)
                for f in fill_post.pop((rep, it, g), ()):
                    f()
                pending.append((rep, it, g, eg))
            # AV lags the scores by AV_LAG groups so it never waits on the
            # in-flight exp (the 1-group lag left only ~40ns of slack)
            done = gg0 + STEP >= NGG * repeat
            while len(pending) > (0 if done else AV_LAG):
                issue_av(*pending.pop(0))

    return nc


def prep_inputs(x, wq, bq, wk, bk, wv, bv):
    """Host-side prep: per-core input maps (numpy)."""
    x = np.asarray(x, dtype=np.float32).reshape(B, C, N)
    wq = np.asarray(wq, dtype=np.float32)
    bq = np.asarray(bq, dtype=np.float32)
    wk = np.asarray(wk, dtype=np.float32)
    wv = np.asarray(wv, dtype=np.float32)
    bv = np.asarray(bv, dtype=np.float32)

    def stack4(w):  # [32, 256] -> [128, 2, 128] (4 copies along cols)
        wT = np.ascontiguousarray(w.T)            # [256, 32]
        out = np.empty((128, 2, 128), dtype=np.float16)
        for kc in range(2):
            out[:, kc, :] = np.tile(wT[128 * kc:128 * (kc + 1)], (1, 4))
        return out

    wq4 = stack4(wq)
    wk4 = stack4(wk)
    bq_h = np.tile(bq, 4)[:, None].astype(np.float32)
    wvT = np.ascontiguousarray(wv.T)              # [256, 256] = [c_in, c_out]
    wvT_h = np.empty((128, 2, C), dtype=np.float16)
    for kc in range(2):
        wvT_h[:, kc, :] = wvT[128 * kc:128 * (kc + 1)]
    shared = dict(wq4=wq4, wk4=wk4, bq=bq_h, wvT=wvT_h)
    return [dict(x=np.ascontiguousarray(x[c]), **shared) for c in range(B)]


@functools.lru_cache(maxsize=4)
def _built_nc(repeat=None):
    return build_nc(repeat)


def run(in_maps, trace=False):
    from concourse.bass_utils import run_bass_kernel_spmd
    nc = _built_nc()
    return run_bass_kernel_spmd(nc, in_maps, core_ids=list(range(N_CORES)),
                                trace=trace)


def kernel(x, wq, bq, wk, bk, wv, bv, _trace=False, _results=None):
    in_maps = prep_inputs(x, wq, bq, wk, bk, wv, bv)
    res = run(in_maps, trace=_trace)
    if _results is not None:
        _results.append(res)
    out = np.stack([np.asarray(res.results[c]["out"], dtype=np.float32).T
                    for c in range(B)])
    out += np.asarray(bv, dtype=np.float32)[None, :, None]
    return out.reshape(B, C, H, W)



# revision 27
# speedup vs baseline: 1.1075x; 1.1075x over previous
"""Trainium2 Bass kernel for nn_AttentionModule (B=8, C=256, HID=32, H=W=64).

Data-parallel over batch: each of the 8 NeuronCores computes one batch
element's full attention:
    q = wq @ xf + bq            [32, 4096]
    k = wk @ xf                 [32, 4096]   (bk dropped: it shifts every
                                 score in a softmax row by the same constant,
                                 so softmax is invariant to it — exact)
    scores^T[j, i] = sum_d k[d, j] q[d, i]
    attn = softmax over j (no max subtraction: |scores| <= ~45, exp fits bf16)
    out[i, c] = (sum_j exp(scoresT[j, i]) vT[j, c]) / (sum_j exp(scoresT[j, i]))

Layout choices:
 - scores^T is computed in [j(partition), i(free)] orientation so that the
   A@V matmul needs no transposes (contraction j on partitions for both
   operands).
 - The softmax denominator comes free as an all-ones 257th column of vT.
 - Output is produced in [i(partition), c(free)] orientation and DMA'd to a
   [N, C] dram tensor; the host transposes back to [C, N] (free on CPU).
 - q/k are replicated 4x along partitions (qstack/kstack [128, 4096]) so the
   K=32 score matmuls can be issued to 4 distinct PE row-groups
   (tile_position) and run concurrently.
 - Exp groups are TGRP=8 j-chunks with a double-buffered PSUM score pool, so
   the PE never waits for the ACT exp of the previous group.
"""

import os
import sys
import math
import functools
from contextlib import ExitStack

import numpy as np

for _p in ("/opt/trn_rl_repo", os.path.expanduser("~/.axon_site/_ro/trn_rl_repo")):
    if os.path.isdir(_p) and _p not in sys.path:
        sys.path.insert(0, _p)

import ml_dtypes  # noqa: E402

import concourse.bass as bass  # noqa: E402
import concourse.tile as tile  # noqa: E402
from concourse import bacc  # noqa: E402
from concourse import mybir  # noqa: E402

B, C, HID, H, W = 8, 256, 32, 64, 64
N = H * W  # 4096
N_CORES = 8

F32 = mybir.dt.float32
F32R = mybir.dt.float32r
BF16 = mybir.dt.bfloat16
F16 = mybir.dt.float16
F8E4 = mybir.dt.float8e4
BF = ml_dtypes.bfloat16

JC = 128                      # j-chunk height (partition dim of scores^T)
N_JCHUNK = N // JC            # 32
TIT = 128                     # i-tile width
N_IT = N // TIT               # 32
TGRP = int(os.environ.get("BASS_TGRP", "8"))   # j-chunks per exp group
TNG = N_JCHUNK // TGRP        # groups per i-tile
PSC_BUFS = int(os.environ.get("BASS_PSC_BUFS", "2"))
POT_BUFS = int(os.environ.get("BASS_POT_BUFS", "2"))
EXPP_BUFS = int(os.environ.get("BASS_EXPP_BUFS", "4"))
AV_LAG = int(os.environ.get("BASS_AV_LAG", "2"))
# score-quad packing: concurrent tile_position matmuls must hit DISTINCT PSUM
# banks — one scp tile [128, 8, 128] spans 2 banks, capping in-tile packing at
# pairs (PACK=2). PACK=4 interleaves TWO groups' score matmuls (their psc
# tiles are different pool buffers = 4 distinct banks total): every adjacent
# window of 4 matmuls covers 4 distinct PE row-quadrants AND 4 distinct PSUM
# banks, without changing the PSUM footprint.
PACK = int(os.environ.get("BASS_PACK", "4"))
# timing probes (break correctness, keep the instruction mix realistic):
#   "scale0"   — exp(0*s)=1: sanity, should time like the real kernel
#   "f8dr"     — scale-0 exp to fp8e4 + DoubleRow AV with main+residual fp8 vT
#   "skipexp"  — exp only on even groups; odd groups reuse the previous eg.
#                PE stream identical, ACT halved: isolates the PE HW factor.
#   "avnarrow" — AV rhs only 128 wide (PE av halved), ACT unchanged:
#                isolates the ACT HW factor.
TPROBE = os.environ.get("BASS_TPROBE", "")
# score-matmul emission order within a pair: "ab8" (group A's 8 matmuls
# first in 2-way bank/quadrant windows, so exp(A)'s semaphore resolves a
# third of a pair earlier — measured 14µs/pass faster than "abab", the
# fully-interleaved 4-way-window order)
ILV = os.environ.get("BASS_ILV", "ab8")
# feed QKV matmuls x directly as bitcast float32r (no f16 conversion pass)
XF32R = os.environ.get("BASS_XF32R", "0") == "1"
# dummy matmuls at t=0 to ramp the PE p-state while waiting for the x DMA
WARMMM = int(os.environ.get("BASS_WARMMM", "0"))


def build_nc(repeat=None):
    nc = _build_inner(repeat)
    nc.compile()
    return nc


def _build_inner(repeat=None):
    """Build the single-core Bass program (run SPMD on 8 cores)."""
    if repeat is None:
        repeat = 1
    nc = bacc.Bacc("TRN2", target_bir_lowering=False, debug=False)

    x_d = nc.dram_tensor("x", [C, N], F32, kind="ExternalInput").ap()
    wq4_d = nc.dram_tensor("wq4", [128, 2, 128], F16,
                           kind="ExternalInput").ap()
    wk4_d = nc.dram_tensor("wk4", [128, 2, 128], F16,
                           kind="ExternalInput").ap()
    bq_d = nc.dram_tensor("bq", [128, 1], F32, kind="ExternalInput").ap()
    wvT_d = nc.dram_tensor("wvT", [128, 2, C], F16, kind="ExternalInput").ap()
    out_d = nc.dram_tensor("out", [N, C], F32, kind="ExternalOutput").ap()

    with tile.TileContext(nc) as tc, ExitStack() as ctx:
        const = ctx.enter_context(tc.tile_pool(name="const", bufs=1))
        stage = ctx.enter_context(tc.tile_pool(name="stage", bufs=1))
        big = ctx.enter_context(tc.tile_pool(name="big", bufs=1))
        expp = ctx.enter_context(tc.tile_pool(name="expp", bufs=EXPP_BUFS))
        outp = ctx.enter_context(tc.tile_pool(name="outp", bufs=3))

        # ---- weights first on sync (small), then x sliced on both queues -----
        # x chunks [128, N] f32, each split into 4 column slices so the first
        # QKV matmul starts after ~1/4 of a chunk lands; f32->f16 conversion
        # per slice alternating DVE/ACT.
        # preload the ACT exp table while the x DMA is in flight (the
        # first real exp would otherwise eat the ~1.3us table load mid-ramp)
        warm = const.tile([1, 1], F32)
        nc.vector.memset(warm, 0.0)
        warme = const.tile([1, 1], BF16)
        nc.scalar.activation(out=warme, in_=warm,
                             func=mybir.ActivationFunctionType.Exp)

        wq4 = const.tile([128, 2, 128], F16)
        nc.sync.dma_start(out=wq4, in_=wq4_d)
        wk4 = const.tile([128, 2, 128], F16)
        nc.sync.dma_start(out=wk4, in_=wk4_d)
        bq = const.tile([128, 1], F32)
        nc.sync.dma_start(out=bq, in_=bq_d)
        wvT = const.tile([128, 2, C], F16)
        nc.gpsimd.dma_start(out=wvT, in_=wvT_d)

        # x [128, N] f32 in column slices over 3 DMA queues, balanced to
        # ~11KB/partition each and ordered by first-use: short head slices
        # so the first QKV matmuls start early, tails cross-assigned so no
        # queue carries more than ~1/3 of the 32KB/partition total.
        SLICES = [(0, 512), (512, 512), (1024, 1024), (2048, 2048)]
        # (ch, si) -> queue: own-chunk queue, slice-1s ride the ACT queue
        QMAP = {(0, 0): "s", (0, 1): "a", (0, 2): "s", (0, 3): "s",
                (1, 0): "g", (1, 1): "a", (1, 2): "g", (1, 3): "g"}
        xqk = None if XF32R else big.tile([128, 2, N], F16, tag="xqk")
        xs_tiles = []
        order = sorted(QMAP, key=lambda t: (t[1], t[0]))
        for ch, si in order:
            off, w = SLICES[si]
            xs = stage.tile([128, w], F32, name=f"xs{ch}_{si}")
            eng = {"s": nc.sync, "g": nc.gpsimd, "a": nc.scalar}[QMAP[(ch, si)]]
            eng.dma_start(
                out=xs,
                in_=x_d[128 * ch:128 * (ch + 1), off:off + w])
            xs_tiles.append((ch, si, off, w, xs))

        def x_ap(ch, col0, width):
            """AP for x[ch-chunk, col0:col0+width] as a matmul operand."""
            if not XF32R:
                return xqk[:, ch, col0:col0 + width]
            for xch, si, off, w, xs in xs_tiles:
                if xch == ch and off <= col0 and col0 + width <= off + w:
                    return xs[:, col0 - off:col0 - off + width].bitcast(F32R)
            raise AssertionError((ch, col0, width))

        if not XF32R:
            # convert slice-major so both chunks' early columns land first;
            # on the otherwise-idle Pool engine (SBUF->SBUF is walrus-safe,
            # unlike PSUM-source gpsimd copies) so ACT/DVE stay free for
            # the qkv copies and early exps during the ramp
            for i, (ch, si, off, w, xs) in enumerate(
                    sorted(xs_tiles, key=lambda t: (t[1], t[0]))):
                sl = slice(off, off + w)
                if i % 2 == 0:
                    nc.vector.tensor_copy(xqk[:, ch, sl], xs)
                else:
                    nc.gpsimd.tensor_copy(xqk[:, ch, sl], xs)

        # ---- qstack/kstack [128, N] (q/k replicated 4x on partitions) --------
        # Only k chunks 0-1 + q chunk 0 are emitted before the main loop; the
        # rest stream in as fillers just ahead of the score/AV matmuls that
        # consume them, so the PE never sits through a serial setup phase.
        qstack = big.tile([128, N], F16, tag="qstack")
        kstack = big.tile([128, N], F16, tag="kstack")
        VW = C + 1
        vT = big.tile([128, N_JCHUNK, VW], BF16, tag="vT")
        nc.vector.memset(vT[:, :, C:C + 1], 1.0)
        pqkv = ctx.enter_context(tc.tile_pool(name="pqkv", bufs=2,
                                              space="PSUM"))

        def emit_k(nch):
            ns = bass.ts(nch, 512)
            pk = pqkv.tile([128, 512], F32, tag="pqkv", name=f"pk{nch}")
            nc.tensor.matmul(pk, lhsT=wk4[:, 0, :],
                             rhs=x_ap(0, 512 * nch, 512),
                             start=True, stop=False)
            nc.tensor.matmul(pk, lhsT=wk4[:, 1, :],
                             rhs=x_ap(1, 512 * nch, 512),
                             start=False, stop=True)
            nc.scalar.activation(out=kstack[:, ns], in_=pk,
                                 func=mybir.ActivationFunctionType.Copy)

        def emit_q(nch):
            ns = bass.ts(nch, 512)
            pq = pqkv.tile([128, 512], F32, tag="pqkv", name=f"pq{nch}")
            nc.tensor.matmul(pq, lhsT=wq4[:, 0, :],
                             rhs=x_ap(0, 512 * nch, 512),
                             start=True, stop=False)
            nc.tensor.matmul(pq, lhsT=wq4[:, 1, :],
                             rhs=x_ap(1, 512 * nch, 512),
                             start=False, stop=True)
            nc.vector.tensor_scalar(out=qstack[:, ns], in0=pq,
                                    scalar1=bq, scalar2=None,
                                    op0=mybir.AluOpType.add)

        def emit_v(jc):
            js = bass.ts(jc, 128)
            pv = pqkv.tile([128, C], F32, tag="pqkv", name=f"pv{jc}")
            nc.tensor.matmul(pv, lhsT=x_ap(0, 128 * jc, 128),
                             rhs=wvT[:, 0, :],
                             start=True, stop=False)
            nc.tensor.matmul(pv, lhsT=x_ap(1, 128 * jc, 128),
                             rhs=wvT[:, 1, :],
                             start=False, stop=True)
            nc.vector.tensor_copy(vT[:, jc, 0:C], pv)

        # PE p-state warm-up: the PE would idle ~3.5us here waiting for x,
        # then run its first ~3us of real matmuls at 0.65/1.2GHz (full clock
        # needs 3us of continuous busy). Dummy matmuls on memset data keep it
        # busy through the DMA window at zero cost.
        if WARMMM:
            warmpe = const.tile([128, 512], F16)
            nc.vector.memset(warmpe, 0.0)
            for wi in range(WARMMM):
                pwarm = pqkv.tile([128, 512], F32, tag="pqkv",
                                  name=f"pwarm{wi}")
                nc.tensor.matmul(pwarm, lhsT=warmpe[:, 0:128], rhs=warmpe,
                                 start=True, stop=True)

        emit_k(0)
        emit_k(1)
        emit_q(0)

        SCHED = [(it, g) for it in range(N_IT) for g in range(TNG)]

        # fillers keyed by (rep, it, g): "pre" run before that group's score
        # matmuls (they feed them), "post" after (they feed the NEXT av)
        fill_pre = {}
        fill_post = {}
        fill_pre[(0, 0, 0)] = [lambda: emit_k(2), lambda: emit_k(3)]
        fill_post[(0, 0, 1)] = [lambda jc=jc: emit_v(jc) for jc in range(0, 8)]
        fill_pre[(0, 0, 2)] = [lambda: emit_k(4), lambda: emit_k(5)]
        fill_post[(0, 0, 2)] = [lambda jc=jc: emit_v(jc) for jc in range(8, 16)]
        fill_pre[(0, 0, 3)] = [lambda: emit_k(6), lambda: emit_k(7)]
        fill_post[(0, 0, 3)] = [lambda jc=jc: emit_v(jc) for jc in range(16, 24)]
        fill_post[(0, 1, 0)] = [lambda jc=jc: emit_v(jc) for jc in range(24, 32)]
        for qc in range(1, 8):
            fill_pre[(0, 4 * qc - 1, 0)] = [lambda qc=qc: emit_q(qc)]

        # ---- fp8 probe tiles --------------------------------------------------
        if TPROBE == "f8dr":
            vT8 = big.tile([128, N_JCHUNK, VW], F8E4, tag="vT8")
            vT8r = big.tile([128, N_JCHUNK, VW], F8E4, tag="vT8r")
            for jc in range(N_JCHUNK):
                nc.vector.tensor_copy(vT8[:, jc, :], vT[:, jc, :])
                nc.vector.tensor_copy(vT8r[:, jc, :], vT[:, jc, :])

        # ---- attention main loop (fused denominator, out in [i, c]) ----------
        psc = ctx.enter_context(tc.tile_pool(name="psc", bufs=PSC_BUFS,
                                             space="PSUM"))
        pot = ctx.enter_context(tc.tile_pool(name="pot", bufs=POT_BUFS,
                                             space="PSUM"))
        pots = {}
        pending = []

        def issue_av(p_rep, p_it, p_g, p_eg):
            p_pot = pots[(p_rep, p_it)]
            if TPROBE == "avnarrow":
                for jcl in range(TGRP):
                    jc = p_g * TGRP + jcl
                    nc.tensor.matmul(p_pot[:, 0:128], lhsT=p_eg[:, jcl, :],
                                     rhs=vT[:, jc, 0:128],
                                     start=jc == 0, stop=jc == N_JCHUNK - 1)
            elif TPROBE == "f8dr":
                for half, vt8x in ((0, vT8), (1, vT8r)):
                    for jp in range(TGRP // 2):
                        jc = p_g * TGRP + 2 * jp
                        nc.tensor.matmul(
                            p_pot,
                            lhsT=p_eg[:, 2 * jp:2 * jp + 2, :],
                            rhs=vt8x[:, jc:jc + 2, :],
                            start=jc == 0 and half == 0,
                            stop=jc == N_JCHUNK - 2 and half == 1,
                            perf_mode=mybir.MatmulPerfMode.DoubleRow)
            else:
                for jcl in range(TGRP):
                    jc = p_g * TGRP + jcl
                    nc.tensor.matmul(p_pot, lhsT=p_eg[:, jcl, :],
                                     rhs=vT[:, jc, :],
                                     start=jc == 0, stop=jc == N_JCHUNK - 1)
            if p_g == TNG - 1:
                isl = bass.ts(p_it, TIT)
                rcp = outp.tile([128, 1], F32, tag="rcp",
                                name=f"rcp{p_rep}_{p_it}")
                nc.vector.reciprocal(rcp, p_pot[:, C:C + 1])
                ots = outp.tile([128, C], F32, tag="ots",
                                name=f"ots{p_rep}_{p_it}")
                nc.vector.tensor_scalar(out=ots, in0=p_pot[:, 0:C],
                                        scalar1=rcp, scalar2=None,
                                        op0=mybir.AluOpType.mult)
                nc.sync.dma_start(out=out_d[isl, :], in_=ots)
                del pots[(p_rep, p_it)]

        def emit_scores(pairs):
            """Score matmuls for one or two groups, interleaved so every
            adjacent window of PACK matmuls hits distinct PE row-quadrants
            and distinct PSUM banks. pairs = [(g, isl, scp), ...]."""
            npair = len(pairs)
            if ILV == "ab8":
                # first group's 8 matmuls complete first (2-way windows),
                # so its exp's semaphore resolves ~1/3 of a pair earlier
                order = [(gi, base, half) for gi in range(npair)
                         for base in range(TGRP // 2) for half in range(2)]
            else:
                order = [(gi, base, half) for base in range(TGRP // 2)
                         for gi in range(npair) for half in range(2)]
            for gi, base, half in order:
                g, isl, scp = pairs[gi]
                jcl = base + half * (TGRP // 2)
                jc = g * TGRP + jcl
                rg = 32 * ((2 * gi + half) % 4)
                nc.tensor.matmul(
                    scp[:, jcl, :],
                    lhsT=kstack[rg:rg + 32, bass.ts(jc, 128)],
                    rhs=qstack[rg:rg + 32, isl],
                    start=True, stop=True,
                    tile_position=(rg, 0))

        def emit_exp(gg, scp):
            if TPROBE == "skipexp" and (gg % 2 == 1) and pending:
                return pending[-1][3]
            eg = expp.tile([128, TGRP, TIT],
                           F8E4 if TPROBE == "f8dr" else BF16, tag="eg")
            nc.scalar.activation(out=eg, in_=scp,
                                 func=mybir.ActivationFunctionType.Exp,
                                 scale=0.0 if TPROBE else 1.0)
            return eg

        NGG = N_IT * TNG
        STEP = 2 if PACK == 4 else 1
        for gg0 in range(0, NGG * repeat + AV_LAG, STEP):
            batch = []
            for gg in range(gg0, min(gg0 + STEP, NGG * repeat)):
                rep, gg_r = divmod(gg, NGG)
                it, g = SCHED[gg_r]
                for f in fill_pre.pop((rep, it, g), ()):
                    f()
                if g == 0:
                    pots[(rep, it)] = pot.tile([128, C + 1], F32, tag="pot",
                                               name=f"pot{rep}_{it}")
                scp = psc.tile([128, TGRP, TIT], F32, tag="scp")
                batch.append((gg, rep, it, g, scp))
            if batch:
                emit_scores([(g, bass.ts(it, TIT), scp)
                             for (gg, rep, it, g, scp) in batch])
            for gg, rep, it, g, scp in batch:
                eg = emit_exp(gg, scp)
                for f in fill_post.pop((rep, it, g), ()):
                    f()
                pending.append((rep, it, g, eg))
            # AV lags the scores by AV_LAG groups so it never waits on the
            # in-flight exp (the 1-group lag left only ~40ns of slack)
            done = gg0 + STEP >= NGG * repeat
            while len(pending) > (0 if done else AV_LAG):
                issue_av(*pending.pop(0))

    return nc


def prep_inputs(x, wq, bq, wk, bk, wv, bv):
    """Host-side prep: per-core input maps (numpy)."""
    x = np.asarray(x, dtype=np.float32).reshape(B, C, N)
    wq = np.asarray(wq, dtype=np.float32)
    bq = np.asarray(bq, dtype=np.float32)
    wk = np.asarray(wk, dtype=np.float32)
    wv = np.asarray(wv, dtype=np.float32)
    bv = np.asarray(bv, dtype=np.float32)

    def stack4(w):  # [32, 256] -> [128, 2, 128] (4 copies along cols)
        wT = np.ascontiguousarray(w.T)            # [256, 32]
        out = np.empty((128, 2, 128), dtype=np.float16)
        for kc in range(2):
            out[:, kc, :] = np.tile(wT[128 * kc:128 * (kc + 1)], (1, 4))
        return out

    wq4 = stack4(wq)
    wk4 = stack4(wk)
    bq_h = np.tile(bq, 4)[:, None].astype(np.float32)
    wvT = np.ascontiguousarray(wv.T)              # [256, 256] = [c_in, c_out]
    wvT_h = np.empty((128, 2, C), dtype=np.float16)
    for kc in range(2):
        wvT_h[:, kc, :] = wvT[128 * kc:128 * (kc + 1)]
    shared = dict(wq4=wq4, wk4=wk4, bq=bq_h, wvT=wvT_h)
    return [dict(x=np.ascontiguousarray(x[c]), **shared) for c in range(B)]


@functools.lru_cache(maxsize=4)
def _built_nc(repeat=None):
    return build_nc(repeat)


def run(in_maps, trace=False):
    from concourse.bass_utils import run_bass_kernel_spmd
    nc = _built_nc()
    return run_bass_kernel_spmd(nc, in_maps, core_ids=list(range(N_CORES)),
                                trace=trace)


def kernel(x, wq, bq, wk, bk, wv, bv, _trace=False, _results=None):
    in_maps = prep_inputs(x, wq, bq, wk, bk, wv, bv)
    res = run(in_maps, trace=_trace)
    if _results is not None:
        _results.append(res)
    out = np.stack([np.asarray(res.results[c]["out"], dtype=np.float32).T
                    for c in range(B)])
    out += np.asarray(bv, dtype=np.float32)[None, :, None]
    return out.reshape(B, C, H, W)

